# revision 3
# baseline (speedup 1.0000x reference)
"""Grouped-experts SwiGLU MoE kernel for Trainium2 (8 NeuronCores).

Expert-parallel: core e computes expert e entirely.
  h = silu(x @ gate) * (x @ down); out = h @ up
Per-core Bass/Tile program (v3, bf16):
  - All matmul operands are bf16 (PSUM accumulation stays fp32). TRN2
    streams bf16 moving operands at 2 cols/cycle, so an N=512 matmul is
    ~107ns and the whole kernel is PE-pair-rate-bound (LDWEIGHTS+MATMUL).
  - x is fed pre-transposed (xT: [D_IN, T]) so the d-contraction matmuls
    read it naturally with d on partitions.
  - Phase 1 produces hT in [j, t] layout (j on partitions), which is exactly
    the stationary layout phase 2 needs -> no on-chip transposes at all.
  - up is loaded into SBUF once per pass (streamed alongside block-0
    phase-1 weights) and stays resident: phase 2 does no DMA-in at all.
  - Output is stored bf16 (host upcasts) to halve the out traffic that
    competes with phase-2 weight streaming.
"""
import sys
if '/opt/trn_rl_repo' not in sys.path:
    sys.path.insert(0, '/opt/trn_rl_repo')
import numpy as np
from concourse import bacc, tile, mybir, bass_utils

E, T, D_IN, D_H = 8, 4096, 2048, 1408
T_B = 1024                 # tokens per block
NK = D_IN // 128           # 16 k-tiles (phase-1 contraction)
NJ = D_H // 128            # 11 j-tiles
NB = T // T_B              # 4 blocks
NC = T_B // 512            # 2 phase-1 moving chunks per block
NTS = T_B // 128           # 8 phase-2 stationary t-subs per block
ND2 = D_IN // 512          # 4 phase-2 output column chunks

BF16_NP = mybir.dt.np(mybir.dt.bfloat16)

_nc_cache = {}


def _build(reps=1):
    f32, bf16 = mybir.dt.float32, mybir.dt.bfloat16
    nc = bacc.Bacc("TRN2", target_bir_lowering=False, debug=False, num_devices=E)
    xT_d = nc.dram_tensor("xT", [D_IN, T], bf16, kind="ExternalInput")
    g_d = nc.dram_tensor("g", [D_IN, D_H], bf16, kind="ExternalInput")
    dn_d = nc.dram_tensor("dn", [D_IN, D_H], bf16, kind="ExternalInput")
    up_d = nc.dram_tensor("up", [D_H, D_IN], bf16, kind="ExternalInput")
    o_d = nc.dram_tensor("o", [T, D_IN], bf16, kind="ExternalOutput")
    Silu = mybir.ActivationFunctionType.Silu

    # strided views for batched DMA
    xT_v = xT_d.ap().rearrange("(k p) t -> p k t", p=128)      # [128, NK, T]
    g_v = g_d.ap().rearrange("(k p) j -> p k j", p=128)        # [128, NK, D_H]
    dn_v = dn_d.ap().rearrange("(k p) j -> p k j", p=128)
    up_v = up_d.ap().rearrange("(j p) d -> p j d", p=128)      # [128, NJ, D_IN]

    with tile.TileContext(nc) as tc:
        with tc.tile_pool(name="sb", bufs=1) as pool, \
             tc.tile_pool(name="xs", bufs=2) as xpool, \
             tc.tile_pool(name="ws", bufs=2) as wpool, \
             tc.tile_pool(name="ps", bufs=2, space="PSUM") as psum:

            def body(first):
                # up stays resident for the whole pass
                ut_full = pool.tile([128, NJ, D_IN], bf16, tag="u")
                for b in range(NB):
                    xt = xpool.tile([128, NK, T_B], bf16, tag="x")
                    gdt0 = None
                    if b == 0 and first:
                        # startup: j=0 weights first, then x in k-quads, so
                        # the first matmuls begin after ~1MB of DMA instead
                        # of ~5MB
                        gt0 = wpool.tile([128, NK, 128], bf16, tag="g")
                        dt0 = wpool.tile([128, NK, 128], bf16, tag="d")
                        gdt0 = (gt0, dt0)
                        nc.sync.dma_start(gt0[:], g_v[:, :, 0:128])
                        nc.sync.dma_start(dt0[:], dn_v[:, :, 0:128])
                        for q in range(4):
                            nc.sync.dma_start(
                                xt[:, q*4:(q+1)*4, 0:512],
                                xT_v[:, q*4:(q+1)*4, 0:512])
                        nc.sync.dma_start(
                            xt[:, :, 512:T_B], xT_v[:, :, 512:T_B])
                    else:
                        for c in range(NC):
                            t0 = b*T_B + c*512
                            nc.sync.dma_start(
                                xt[:, :, c*512:(c+1)*512], xT_v[:, :, t0:t0+512])
                    hts = []
                    for j in range(NJ):
                        if j == 0 and gdt0 is not None:
                            gt, dt = gdt0
                        else:
                            gt = wpool.tile([128, NK, 128], bf16, tag="g")
                            dt = wpool.tile([128, NK, 128], bf16, tag="d")
                            nc.sync.dma_start(gt[:], g_v[:, :, j*128:(j+1)*128])
                            nc.sync.dma_start(dt[:], dn_v[:, :, j*128:(j+1)*128])
                        if b == 0:
                            # stream up's j-th panel alongside block-0 phase-1
                            # weights; resident for the rest of the pass
                            nc.sync.dma_start(
                                ut_full[:, j, :], up_v[:, j, :])
                        ht = pool.tile([128, T_B], bf16, tag=f"h{j}")
                        for c in range(NC):
                            pg = psum.tile([128, 512], f32, tag="pg")
                            pd = psum.tile([128, 512], f32, tag="pd")
                            xs = [xt[:, k, c*512:(c+1)*512] for k in range(NK)]
                            for k in range(NK):
                                nc.tensor.matmul(pg[:], gt[:, k, :], xs[k],
                                                 start=(k == 0), stop=(k == NK-1))
                            for k in range(NK):
                                nc.tensor.matmul(pd[:], dt[:, k, :], xs[k],
                                                 start=(k == 0), stop=(k == NK-1))
                            tmp = wpool.tile([128, 512], f32, tag="silu")
                            nc.scalar.activation(tmp[:], pg[:], Silu)
                            nc.vector.tensor_mul(
                                ht[:, c*512:(c+1)*512], tmp[:], pd[:])
                        hts.append(ht)
                    for dc in range(ND2):
                        for ts in range(NTS):
                            po = psum.tile([128, 512], f32, tag="po")
                            for j in range(NJ):
                                nc.tensor.matmul(
                                    po[:], hts[j][:, ts*128:(ts+1)*128],
                                    ut_full[:, j, dc*512:(dc+1)*512],
                                    start=(j == 0), stop=(j == NJ-1))
                            ot = wpool.tile([128, 512], bf16, tag="ot")
                            nc.any.tensor_copy(ot[:], po[:])
                            r0 = b*T_B + ts*128
                            nc.sync.dma_start(
                                o_d.ap()[r0:r0+128, dc*512:(dc+1)*512], ot[:])

            if reps == 1:
                body(first=True)
            else:
                with tc.For_i(0, reps):
                    body(first=False)
    nc.compile()
    return nc


def _get_nc(reps=1):
    if reps not in _nc_cache:
        _nc_cache[reps] = _build(reps)
    return _nc_cache[reps]


def kernel(x, gate_proj, down_proj, up_proj, tokens_per_expert):
    x = np.asarray(x, dtype=np.float32)
    gate_proj = np.asarray(gate_proj, dtype=np.float32)
    down_proj = np.asarray(down_proj, dtype=np.float32)
    up_proj = np.asarray(up_proj, dtype=np.float32)
    nc = _get_nc()
    in_maps = [{
        "xT": np.ascontiguousarray(x[e].T).astype(BF16_NP),
        "g": np.ascontiguousarray(gate_proj[e]).astype(BF16_NP),
        "dn": np.ascontiguousarray(down_proj[e]).astype(BF16_NP),
        "up": np.ascontiguousarray(up_proj[e]).astype(BF16_NP),
    } for e in range(E)]
    res = bass_utils.run_bass_kernel_spmd(nc, in_maps, list(range(E)))
    return np.stack([res.results[e]["o"] for e in range(E)], axis=0).astype(np.float32)


# revision 20
# speedup vs baseline: 1.9830x; 1.9830x over previous
"""Grouped-experts SwiGLU MoE kernel for Trainium2 (8 NeuronCores).

Expert-parallel: core e computes expert e entirely.
  h = silu(x @ gate) * (x @ down); out = h @ up
Per-core Bass/Tile program (bf16):
  - All matmul operands are bf16 (PSUM accumulation stays fp32). TRN2
    streams bf16 moving operands at 2 cols/cycle, so an N=512 matmul is
    ~107ns and the whole kernel is PE-pair-rate-bound (LDWEIGHTS+MATMUL).
  - x is fed pre-transposed (xT: [D_IN, T]) so the d-contraction matmuls
    read it naturally with d on partitions.
  - Phase 1 produces hT in [j, t] layout (j on partitions), which is exactly
    the stationary layout phase 2 needs -> no on-chip transposes at all.
"""
import sys
if '/opt/trn_rl_repo' not in sys.path:
    sys.path.insert(0, '/opt/trn_rl_repo')
import numpy as np
from concourse import bacc, tile, mybir, bass_utils

E, T, D_IN, D_H = 8, 4096, 2048, 1408
NK = D_IN // 128           # 16 k-tiles (phase-1 contraction)
NJ = D_H // 128            # 11 j-tiles
ND2 = D_IN // 512          # 4 phase-2 output column chunks

BF16_NP = mybir.dt.np(mybir.dt.bfloat16)

# default config = current best (burst-regime measured: ~933us/core)
CFG = dict(u_resident=False, out_bf16=True, x_bufs=1, po_bufs=3,
           out_q="sync", w_bufs=3, t_b=1024)

_nc_cache = {}


def _build(reps=1, **over):
    cfg = dict(CFG)
    cfg.update(over)
    u_resident = cfg["u_resident"]
    out_bf16 = cfg["out_bf16"]
    x_bufs = cfg["x_bufs"]
    po_bufs = cfg["po_bufs"]
    out_q = cfg["out_q"]
    w_bufs = cfg["w_bufs"]
    delay_us = cfg.get("delay_us", 0)
    delay_only = cfg.get("delay_only", False)
    T_B = cfg["t_b"]           # tokens per block
    NB = T // T_B              # blocks
    NC = T_B // 512            # phase-1 moving chunks per block
    NTS = T_B // 128           # phase-2 stationary t-subs per block

    f32, bf16 = mybir.dt.float32, mybir.dt.bfloat16
    o_dt = bf16 if out_bf16 else f32
    nc = bacc.Bacc("TRN2", target_bir_lowering=False, debug=False, num_devices=E)
    xT_d = nc.dram_tensor("xT", [D_IN, T], bf16, kind="ExternalInput")
    g_d = nc.dram_tensor("g", [D_IN, D_H], bf16, kind="ExternalInput")
    dn_d = nc.dram_tensor("dn", [D_IN, D_H], bf16, kind="ExternalInput")
    up_d = nc.dram_tensor("up", [D_H, D_IN], bf16, kind="ExternalInput")
    o_d = nc.dram_tensor("o", [T, D_IN], o_dt, kind="ExternalOutput")
    Silu = mybir.ActivationFunctionType.Silu

    # strided views for batched DMA
    xT_v = xT_d.ap().rearrange("(k p) t -> p k t", p=128)      # [128, NK, T]
    g_v = g_d.ap().rearrange("(k p) j -> p k j", p=128)        # [128, NK, D_H]
    dn_v = dn_d.ap().rearrange("(k p) j -> p k j", p=128)
    up_v = up_d.ap().rearrange("(j p) d -> p j d", p=128)      # [128, NJ, D_IN]

    with tile.TileContext(nc) as tc:
        with tc.tile_pool(name="sb", bufs=1) as pool, \
             tc.tile_pool(name="xs", bufs=x_bufs) as xpool, \
             tc.tile_pool(name="ws", bufs=w_bufs) as wpool, \
             tc.tile_pool(name="ps", bufs=2, space="PSUM") as psum, \
             tc.tile_pool(name="ps2", bufs=po_bufs, space="PSUM") as psum2, \
             tc.tile_pool(name="psf", bufs=1, space="PSUM") as psumf:
            out_eng = nc.scalar if out_q == "act" else nc.sync

            def body(first):
                if delay_us:
                    # Device-side cooling delay: a serial (data-dependent)
                    # chain of DVE copies burns a fixed, throttle-independent
                    # time (DVE has no HAM), then a 1-col "fence" matmul
                    # reads the chain output so every real matmul queues
                    # behind it on the PE FIFO. Lets each For_i rep run in
                    # the burst regime (chip cools between reps).
                    d0 = pool.tile([128, 8192], bf16, tag="dly0")
                    d1 = pool.tile([128, 8192], bf16, tag="dly1")
                    nc.vector.memset(d0[:], 0.0)
                    n_ops = max(2, int(delay_us / 2.93)) & ~1
                    for i in range(n_ops):
                        src, dst = (d0, d1) if i % 2 == 0 else (d1, d0)
                        nc.vector.tensor_copy(dst[:], src[:])
                    # hard fence: nothing (matmuls OR DMAs) may be scheduled
                    # before the delay completes -> each rep is a faithful
                    # cold start in the burst regime
                    tc.strict_bb_all_engine_barrier()
                if delay_only:
                    return
                ut_full = None
                if u_resident:
                    ut_full = pool.tile([128, NJ, D_IN], bf16, tag="u")
                for b in range(NB):
                    xt = xpool.tile([128, NK, T_B], bf16, tag="x")
                    gdt0 = None
                    if b == 0 and first:
                        # startup: j=0 weights first, then x in k-quads, so
                        # the first matmuls begin after ~1MB of DMA instead
                        # of ~5MB
                        gt0 = wpool.tile([128, NK, 128], bf16, tag="g")
                        dt0 = wpool.tile([128, NK, 128], bf16, tag="d")
                        gdt0 = (gt0, dt0)
                        nc.sync.dma_start(gt0[:], g_v[:, :, 0:128])
                        nc.sync.dma_start(dt0[:], dn_v[:, :, 0:128])
                        for q in range(4):
                            nc.sync.dma_start(
                                xt[:, q*4:(q+1)*4, 0:512],
                                xT_v[:, q*4:(q+1)*4, 0:512])
                        nc.sync.dma_start(
                            xt[:, :, 512:T_B], xT_v[:, :, 512:T_B])
                    else:
                        for c in range(NC):
                            t0 = b*T_B + c*512
                            nc.sync.dma_start(
                                xt[:, :, c*512:(c+1)*512], xT_v[:, :, t0:t0+512])
                    hts = []
                    for j in range(NJ):
                        if j == 0 and gdt0 is not None:
                            gt, dt = gdt0
                        else:
                            gt = wpool.tile([128, NK, 128], bf16, tag="g")
                            dt = wpool.tile([128, NK, 128], bf16, tag="d")
                            nc.sync.dma_start(gt[:], g_v[:, :, j*128:(j+1)*128])
                            nc.sync.dma_start(dt[:], dn_v[:, :, j*128:(j+1)*128])
                        if u_resident and b == 0:
                            nc.sync.dma_start(
                                ut_full[:, j, :], up_v[:, j, :])
                        ht = pool.tile([128, T_B], bf16, tag=f"h{j}")
                        for c in range(NC):
                            pg = psum.tile([128, 512], f32, tag="pg")
                            pd = psum.tile([128, 512], f32, tag="pd")
                            xs = [xt[:, k, c*512:(c+1)*512] for k in range(NK)]
                            for k in range(NK):
                                nc.tensor.matmul(pg[:], gt[:, k, :], xs[k],
                                                 start=(k == 0), stop=(k == NK-1))
                            for k in range(NK):
                                nc.tensor.matmul(pd[:], dt[:, k, :], xs[k],
                                                 start=(k == 0), stop=(k == NK-1))
                            tmp = wpool.tile([128, 512], f32, tag="silu")
                            nc.scalar.activation(tmp[:], pg[:], Silu)
                            nc.vector.tensor_mul(
                                ht[:, c*512:(c+1)*512], tmp[:], pd[:])
                        hts.append(ht)
                    for dc in range(ND2):
                        uts = []
                        if not u_resident:
                            for j in range(NJ):
                                ut = wpool.tile([128, 512], bf16, tag=f"u{j}")
                                nc.sync.dma_start(
                                    ut[:], up_v[:, j, dc*512:(dc+1)*512])
                                uts.append(ut)
                        for ts in range(NTS):
                            po = psum2.tile([128, 512], f32, tag="po")
                            for j in range(NJ):
                                rhs = (ut_full[:, j, dc*512:(dc+1)*512]
                                       if u_resident else uts[j][:])
                                nc.tensor.matmul(
                                    po[:], hts[j][:, ts*128:(ts+1)*128], rhs,
                                    start=(j == 0), stop=(j == NJ-1))
                            ot = wpool.tile([128, 512], o_dt, tag="ot")
                            nc.any.tensor_copy(ot[:], po[:])
                            r0 = b*T_B + ts*128
                            out_eng.dma_start(
                                o_d.ap()[r0:r0+128, dc*512:(dc+1)*512], ot[:])

            if reps == 1:
                body(first=True)
            else:
                with tc.For_i(0, reps):
                    body(first=False)
    nc.compile()
    return nc


def _get_nc(reps=1, **over):
    key = (reps,) + tuple(sorted(over.items()))
    if key not in _nc_cache:
        _nc_cache[key] = _build(reps, **over)
    return _nc_cache[key]


def kernel(x, gate_proj, down_proj, up_proj, tokens_per_expert):
    x = np.asarray(x, dtype=np.float32)
    gate_proj = np.asarray(gate_proj, dtype=np.float32)
    down_proj = np.asarray(down_proj, dtype=np.float32)
    up_proj = np.asarray(up_proj, dtype=np.float32)
    nc = _get_nc()
    in_maps = [{
        "xT": np.ascontiguousarray(x[e].T).astype(BF16_NP),
        "g": np.ascontiguousarray(gate_proj[e]).astype(BF16_NP),
        "dn": np.ascontiguousarray(down_proj[e]).astype(BF16_NP),
        "up": np.ascontiguousarray(up_proj[e]).astype(BF16_NP),
    } for e in range(E)]
    res = bass_utils.run_bass_kernel_spmd(nc, in_maps, list(range(E)))
    return np.stack([res.results[e]["o"] for e in range(E)], axis=0).astype(np.float32)


# revision 22
# speedup vs baseline: 1.9975x; 1.0073x over previous
"""Grouped-experts SwiGLU MoE kernel for Trainium2 (8 NeuronCores).

Expert-parallel: core e computes expert e entirely.
  h = silu(x @ gate) * (x @ down); out = h @ up
Per-core Bass/Tile program (bf16):
  - All matmul operands are bf16 (PSUM accumulation stays fp32). TRN2
    streams bf16 moving operands at 2 cols/cycle, so an N=512 matmul is
    ~107ns and the whole kernel is PE-pair-rate-bound (LDWEIGHTS+MATMUL).
  - x is fed pre-transposed (xT: [D_IN, T]) so the d-contraction matmuls
    read it naturally with d on partitions.
  - Phase 1 produces hT in [j, t] layout (j on partitions), which is exactly
    the stationary layout phase 2 needs -> no on-chip transposes at all.
"""
import sys
if '/opt/trn_rl_repo' not in sys.path:
    sys.path.insert(0, '/opt/trn_rl_repo')
import numpy as np
from concourse import bacc, tile, mybir, bass_utils

E, T, D_IN, D_H = 8, 4096, 2048, 1408
NK = D_IN // 128           # 16 k-tiles (phase-1 contraction)
NJ = D_H // 128            # 11 j-tiles
ND2 = D_IN // 512          # 4 phase-2 output column chunks

BF16_NP = mybir.dt.np(mybir.dt.bfloat16)

# default config = current best (8-core burst-regime measured: ~958us/core)
CFG = dict(u_resident=False, out_bf16=True, x_bufs=2, po_bufs=3,
           out_q="sync", w_bufs=3, t_b=1024)

_nc_cache = {}


def _build(reps=1, **over):
    cfg = dict(CFG)
    cfg.update(over)
    u_resident = cfg["u_resident"]
    out_bf16 = cfg["out_bf16"]
    x_bufs = cfg["x_bufs"]
    po_bufs = cfg["po_bufs"]
    out_q = cfg["out_q"]
    w_bufs = cfg["w_bufs"]
    delay_us = cfg.get("delay_us", 0)
    delay_only = cfg.get("delay_only", False)
    T_B = cfg["t_b"]           # tokens per block
    NB = T // T_B              # blocks
    NC = T_B // 512            # phase-1 moving chunks per block
    NTS = T_B // 128           # phase-2 stationary t-subs per block

    f32, bf16 = mybir.dt.float32, mybir.dt.bfloat16
    o_dt = bf16 if out_bf16 else f32
    nc = bacc.Bacc("TRN2", target_bir_lowering=False, debug=False, num_devices=E)
    xT_d = nc.dram_tensor("xT", [D_IN, T], bf16, kind="ExternalInput")
    g_d = nc.dram_tensor("g", [D_IN, D_H], bf16, kind="ExternalInput")
    dn_d = nc.dram_tensor("dn", [D_IN, D_H], bf16, kind="ExternalInput")
    up_d = nc.dram_tensor("up", [D_H, D_IN], bf16, kind="ExternalInput")
    o_d = nc.dram_tensor("o", [T, D_IN], o_dt, kind="ExternalOutput")
    Silu = mybir.ActivationFunctionType.Silu

    # strided views for batched DMA
    xT_v = xT_d.ap().rearrange("(k p) t -> p k t", p=128)      # [128, NK, T]
    g_v = g_d.ap().rearrange("(k p) j -> p k j", p=128)        # [128, NK, D_H]
    dn_v = dn_d.ap().rearrange("(k p) j -> p k j", p=128)
    up_v = up_d.ap().rearrange("(j p) d -> p j d", p=128)      # [128, NJ, D_IN]

    with tile.TileContext(nc) as tc:
        with tc.tile_pool(name="sb", bufs=1) as pool, \
             tc.tile_pool(name="xs", bufs=x_bufs) as xpool, \
             tc.tile_pool(name="ws", bufs=w_bufs) as wpool, \
             tc.tile_pool(name="ps", bufs=2, space="PSUM") as psum, \
             tc.tile_pool(name="ps2", bufs=po_bufs, space="PSUM") as psum2, \
             tc.tile_pool(name="psf", bufs=1, space="PSUM") as psumf:
            out_eng = nc.scalar if out_q == "act" else nc.sync

            def body(first):
                if delay_us:
                    # Device-side cooling delay: a serial (data-dependent)
                    # chain of DVE copies burns a fixed, throttle-independent
                    # time (DVE has no HAM), then a 1-col "fence" matmul
                    # reads the chain output so every real matmul queues
                    # behind it on the PE FIFO. Lets each For_i rep run in
                    # the burst regime (chip cools between reps).
                    d0 = pool.tile([128, 8192], bf16, tag="dly0")
                    d1 = pool.tile([128, 8192], bf16, tag="dly1")
                    nc.vector.memset(d0[:], 0.0)
                    n_ops = max(2, int(delay_us / 2.93)) & ~1
                    for i in range(n_ops):
                        src, dst = (d0, d1) if i % 2 == 0 else (d1, d0)
                        nc.vector.tensor_copy(dst[:], src[:])
                    # hard fence: nothing (matmuls OR DMAs) may be scheduled
                    # before the delay completes -> each rep is a faithful
                    # cold start in the burst regime
                    tc.strict_bb_all_engine_barrier()
                if delay_only:
                    return
                ut_full = None
                if u_resident:
                    ut_full = pool.tile([128, NJ, D_IN], bf16, tag="u")
                for b in range(NB):
                    xt = xpool.tile([128, NK, T_B], bf16, tag="x")
                    gdt0 = None
                    if b == 0 and first:
                        # startup: j=0 weights first, then x in k-quads, so
                        # the first matmuls begin after ~1MB of DMA instead
                        # of ~5MB
                        gt0 = wpool.tile([128, NK, 128], bf16, tag="g")
                        dt0 = wpool.tile([128, NK, 128], bf16, tag="d")
                        gdt0 = (gt0, dt0)
                        nc.sync.dma_start(gt0[:], g_v[:, :, 0:128])
                        nc.sync.dma_start(dt0[:], dn_v[:, :, 0:128])
                        for q in range(4):
                            nc.sync.dma_start(
                                xt[:, q*4:(q+1)*4, 0:512],
                                xT_v[:, q*4:(q+1)*4, 0:512])
                        nc.sync.dma_start(
                            xt[:, :, 512:T_B], xT_v[:, :, 512:T_B])
                    else:
                        for c in range(NC):
                            t0 = b*T_B + c*512
                            nc.sync.dma_start(
                                xt[:, :, c*512:(c+1)*512], xT_v[:, :, t0:t0+512])
                    hts = []
                    for j in range(NJ):
                        if j == 0 and gdt0 is not None:
                            gt, dt = gdt0
                        else:
                            gt = wpool.tile([128, NK, 128], bf16, tag="g")
                            dt = wpool.tile([128, NK, 128], bf16, tag="d")
                            nc.sync.dma_start(gt[:], g_v[:, :, j*128:(j+1)*128])
                            nc.sync.dma_start(dt[:], dn_v[:, :, j*128:(j+1)*128])
                        if u_resident and b == 0:
                            nc.sync.dma_start(
                                ut_full[:, j, :], up_v[:, j, :])
                        ht = pool.tile([128, T_B], bf16, tag=f"h{j}")
                        for c in range(NC):
                            pg = psum.tile([128, 512], f32, tag="pg")
                            pd = psum.tile([128, 512], f32, tag="pd")
                            xs = [xt[:, k, c*512:(c+1)*512] for k in range(NK)]
                            for k in range(NK):
                                nc.tensor.matmul(pg[:], gt[:, k, :], xs[k],
                                                 start=(k == 0), stop=(k == NK-1))
                            for k in range(NK):
                                nc.tensor.matmul(pd[:], dt[:, k, :], xs[k],
                                                 start=(k == 0), stop=(k == NK-1))
                            tmp = wpool.tile([128, 512], f32, tag="silu")
                            nc.scalar.activation(tmp[:], pg[:], Silu)
                            nc.vector.tensor_mul(
                                ht[:, c*512:(c+1)*512], tmp[:], pd[:])
                        hts.append(ht)
                    for dc in range(ND2):
                        uts = []
                        if not u_resident:
                            for j in range(NJ):
                                ut = wpool.tile([128, 512], bf16, tag=f"u{j}")
                                nc.sync.dma_start(
                                    ut[:], up_v[:, j, dc*512:(dc+1)*512])
                                uts.append(ut)
                        for ts in range(NTS):
                            po = psum2.tile([128, 512], f32, tag="po")
                            for j in range(NJ):
                                rhs = (ut_full[:, j, dc*512:(dc+1)*512]
                                       if u_resident else uts[j][:])
                                nc.tensor.matmul(
                                    po[:], hts[j][:, ts*128:(ts+1)*128], rhs,
                                    start=(j == 0), stop=(j == NJ-1))
                            ot = wpool.tile([128, 512], o_dt, tag="ot")
                            nc.any.tensor_copy(ot[:], po[:])
                            r0 = b*T_B + ts*128
                            out_eng.dma_start(
                                o_d.ap()[r0:r0+128, dc*512:(dc+1)*512], ot[:])

            if reps == 1:
                body(first=True)
            else:
                with tc.For_i(0, reps):
                    body(first=True)
    nc.compile()
    return nc


def _get_nc(reps=1, **over):
    key = (reps,) + tuple(sorted(over.items()))
    if key not in _nc_cache:
        _nc_cache[key] = _build(reps, **over)
    return _nc_cache[key]


def kernel(x, gate_proj, down_proj, up_proj, tokens_per_expert):
    x = np.asarray(x, dtype=np.float32)
    gate_proj = np.asarray(gate_proj, dtype=np.float32)
    down_proj = np.asarray(down_proj, dtype=np.float32)
    up_proj = np.asarray(up_proj, dtype=np.float32)
    nc = _get_nc()
    in_maps = [{
        "xT": np.ascontiguousarray(x[e].T).astype(BF16_NP),
        "g": np.ascontiguousarray(gate_proj[e]).astype(BF16_NP),
        "dn": np.ascontiguousarray(down_proj[e]).astype(BF16_NP),
        "up": np.ascontiguousarray(up_proj[e]).astype(BF16_NP),
    } for e in range(E)]
    res = bass_utils.run_bass_kernel_spmd(nc, in_maps, list(range(E)))
    return np.stack([res.results[e]["o"] for e in range(E)], axis=0).astype(np.float32)
